# revision 39
# baseline (speedup 1.0000x reference)
"""Trainium2 Bass kernel for Mobile2Former cross-attention block.

Computation (per batch b):
    xf   = x[b].reshape(C, H*W)                      # [64, 3136] keys=values
    q    = (z[b] @ Wq + bq).reshape(heads, M, C)     # [8, 6, 64]
    attn = softmax(q @ xf * C**-0.5, axis=-1)        # [8, 6, 3136]
    res  = attn @ xf.T                               # [8, 6, 64]
    out  = res.transpose(1,0,2).reshape(M, -1) @ Wo + bo + z[b]

Strategy: data-parallel over B across 8 cores (16 batches/core), batches
processed in pairs (two batches stacked on the 128 SBUF partitions, C=64
each).  QK^T is computed directly in transposed layout (attn^T[n, hm]) by
using xf chunks as the matmul stationary operand.  The AV matmul consumes
x^T chunks that are pre-transposed on the HOST and DMAed directly (fp8),
eliminating all PE transposes.  Both x layouts ship as fp8e4 so total DMA
bytes match a single bf16 copy.  The softmax scale is applied via the
activation instruction's free affine; exp runs as ONE big ACTIVATE per
pair reading a bf16 SBUF staging copy of the logits (5 cheap DVE
copies/pair drain PSUM).  Softmax denominators come from a ones-column
baked into the host x^T image.  The ragged last spatial chunk (3136 =
24.5*128) is handled by overlapping chunk 24 with chunk 23 (covers
n=3008:3136) and zeroing the duplicated rows in the host image, keeping
every matmul/activation a full rectangle.  Each pair's AV is deferred one
loop iteration so its exp overlaps the next pair's QK on the PE.
"""

import sys
from contextlib import ExitStack

import numpy as np

sys.path.insert(0, "/opt/trn_rl_repo")

import concourse.bass as bass
import concourse.tile as tile
from concourse import bacc as bacc_mod
from concourse import mybir
from concourse.bass_utils import run_bass_kernel_spmd

try:
    import ml_dtypes

    BF16 = ml_dtypes.bfloat16
    F8 = ml_dtypes.float8_e4m3fn
except ImportError:  # pragma: no cover
    import jax.numpy as jnp

    BF16 = jnp.bfloat16
    F8 = jnp.float8_e4m3fn

N_CORES = 8
B, C, H, W = 128, 64, 56, 56
HW = H * W  # 3136
M, D = 6, 192
NH = 8
INNER = NH * C  # 512
BPC = B // N_CORES  # 16 batches per core
NPAIR = BPC // 2  # 8 pairs per core
NCHUNK = 25  # 24 full 128-chunks + one OVERLAPPED chunk (n 3008:3136)
SCALE = float(C) ** -0.5

F32 = mybir.dt.float32
BF = mybir.dt.bfloat16
FP8 = mybir.dt.float8e4

_CACHE = {}


def _build_nc() -> bass.Bass:
    nc = bacc_mod.Bacc()

    # x in both layouts packed per pair-row: [xf 3136 | x^T-chunks 25*129],
    # the x^T blocks carry a ones-column at col 128 (softmax denominator).
    XROW = HW + 129 * NCHUNK  # 6361
    xx8_h = nc.declare_dram_parameter("xx8", [NPAIR, 128, XROW], FP8, isOutput=False)
    # host-computed block-diagonal q^T images, all pairs: [q, 96p + 48b + u]
    qt8_h = nc.declare_dram_parameter("qt8", [128, NPAIR * 96], FP8, isOutput=False)
    # zbo/out as [96, 192]: row = 12*pair + 6*b + m = batch-major (b, m)
    zbo_h = nc.declare_dram_parameter("zbo", [BPC * M, D], F32, isOutput=False)
    # pk2 cols: [ident_bf 128][wo 4*192]
    pk2_h = nc.declare_dram_parameter("pk2", [128, 896], BF, isOutput=False)
    out_h = nc.declare_dram_parameter("out", [BPC * M, D], F32, isOutput=True)

    xx8_r = xx8_h.ap().rearrange("p q n -> (p q) n")  # [1024, 6361]

    with tile.TileContext(nc) as tc, ExitStack() as ctx:
        const = ctx.enter_context(tc.tile_pool(name="const", bufs=1))
        xx_pool = ctx.enter_context(tc.tile_pool(name="xx", bufs=NPAIR))
        small = ctx.enter_context(tc.tile_pool(name="small", bufs=3))
        # logit PSUM: wave A = chunks 0-14 (3 banks), wave B = 15-24 (2 banks)
        atA_psum = ctx.enter_context(tc.tile_pool(name="atA_ps", bufs=1, space="PSUM"))
        atB_psum = ctx.enter_context(tc.tile_pool(name="atB_ps", bufs=1, space="PSUM"))
        rs_psum = ctx.enter_context(tc.tile_pool(name="rs_ps", bufs=1, space="PSUM"))
        sm_psum = ctx.enter_context(tc.tile_pool(name="sm_ps", bufs=1, space="PSUM"))
        o2_psum = ctx.enter_context(tc.tile_pool(name="o2_ps", bufs=1, space="PSUM"))

        # ---------------- phase 0: constants ----------------
        qt_sb = const.tile([128, NPAIR * 96], FP8)
        nc.sync.dma_start(out=qt_sb, in_=qt8_h.ap())
        # pk2/zbo are loaded on the ACT ring AFTER pair 0's x tiles (below).
        pk2 = const.tile([128, 896], BF)
        ident_bf = pk2[:, 0:128]
        wo_sb = pk2[:, 128:896]
        zbo_sb = const.tile([BPC * M, D], F32)
        warm = const.tile([1, 1], F32)
        nc.gpsimd.memset(warm, 0.0)
        warm2 = const.tile([1, 1], F32)

        # Persistent fp8 E buffers: cols 96:128 of each 128-block are static
        # zeros (pads the AV stationary to 128 cols for fast weight load).
        ax_bufs = []
        for i in range(2):
            t = const.tile([128, NCHUNK * 128], FP8, name=f"ax_buf{i}")
            tv = t.rearrange("q (j c) -> q j c", j=NCHUNK)
            nc.gpsimd.memset(tv[:, :, 96:128], 0.0)
            ax_bufs.append(t)
        # PE/HAM warm-up on junk zeros while the first DMAs are in flight
        # (targets the wave-B slot so pair 0's wave A is never blocked).
        warm_ps = atB_psum.tile([128, 1024], F32, tag="at15", name="warm_ps")
        for wmi in range(4):
            nc.tensor.matmul(
                warm_ps[:, 0:512], lhsT=ax_bufs[0][:, 0:128],
                rhs=ax_bufs[0][:, 0:512], start=True, stop=True,
            )

        # ---------------- per-pair software pipeline ----------------
        # column order inside a pair: hm2 = 48*b + u, u = 6*h + m (the q
        # images, incl. the block-diagonal zeros, are prebuilt on host).
        # normalized-attention output staged across pairs for one batched
        # Wo projection at the end: fcl_all[q, kk, 12p + 6b + m]
        fcl_all = const.tile([128, 4 * 96], BF)
        fcl_all_g = fcl_all.rearrange("q (kk x) -> q kk x", kk=4)
        # All x tiles resident; one need-ordered stream of descriptors on
        # the SP ring (deep outstanding queue sustains ~400 B/ns).  Pair 0/1
        # xf pieces are fine-grained so the first QK waves chase the DMA.
        xx_tiles = []
        for p in range(NPAIR):
            xx_tiles.append(
                xx_pool.tile([128, XROW], FP8, tag="xx", name=f"xx{p}")
            )

        def _ld(p, c0, c1):
            nc.sync.dma_start(
                out=xx_tiles[p][:, c0:c1],
                in_=xx8_r[128 * p : 128 * (p + 1), c0:c1],
            )

        _ld(0, 0, 640)
        _ld(0, 640, 1920)
        _ld(0, 1920, HW)
        _ld(1, 0, 1920)
        _ld(0, HW, XROW)
        _ld(1, 1920, HW)
        _ld(2, 0, HW)
        _ld(1, HW, XROW)
        _ld(3, 0, HW)
        _ld(2, HW, XROW)
        nc.sync.dma_start(out=pk2, in_=pk2_h.ap())
        nc.sync.dma_start(out=zbo_sb, in_=zbo_h.ap())
        _ld(4, 0, HW)
        _ld(3, HW, XROW)
        _ld(5, 0, HW)
        _ld(4, HW, XROW)
        _ld(6, 0, HW)
        _ld(5, HW, XROW)
        _ld(7, 0, HW)
        _ld(6, HW, XROW)
        _ld(7, HW, XROW)

        def emit_front(p):
            """DMA + q assembly + QK + exp for pair p."""
            xx = xx_tiles[p]
            xf = xx[:, 0:HW]
            if p == 0:
                # ACT spline-table prewarm ahead of the first real exp (the
                # ACT queue carries ONLY activations; DMA issues would stall
                # behind the exp chain).
                nc.scalar.activation(
                    out=warm2, in_=warm, func=mybir.ActivationFunctionType.Exp
                )

            qT2 = qt_sb[:, 96 * p : 96 * p + 96]
            ax = ax_bufs[p % 2]
            # waves: A = chunks 0-14 (3 PSUM banks), B = 15-24 (2 banks);
            # 5 chunks per 512-f32 bank (96*5=480 + 32 pad, no bank crossing)
            for nb, j0, pool in ((3, 0, atA_psum), (2, 15, atB_psum)):
                at = pool.tile([128, 512 * nb], F32, tag=f"at{j0}", name=f"at{p}_{j0}")
                for jj in range(5 * nb):
                    j = j0 + jj
                    n0 = 128 * j if j < NCHUNK - 1 else HW - 128
                    o = 512 * (jj // 5) + 96 * (jj % 5)
                    nc.tensor.matmul(
                        out=at[:, o : o + 96],
                        lhsT=xf[:, n0 : n0 + 128], rhs=qT2,
                        start=True, stop=True,
                    )
                # one big exp per wave, strided 4D APs skipping the bank pads
                in4 = bass.AP(
                    tensor=at.tensor, offset=at.offset,
                    ap=[list(at.ap[0]), [512, nb], [96, 5], [1, 96]],
                )
                out4 = bass.AP(
                    tensor=ax.tensor, offset=ax.offset + 128 * j0,
                    ap=[list(ax.ap[0]), [640, nb], [128, 5], [1, 96]],
                )
                nc.scalar.activation(
                    out=out4, in_=in4,
                    func=mybir.ActivationFunctionType.Exp, scale=SCALE,
                )

        r2n_tiles = [None] * NPAIR

        def emit_av(p):
            """AV + softmax normalization for pair p."""
            ax = ax_bufs[p % 2]
            xts = xx_tiles[p]
            rsum = rs_psum.tile([128, 129], F32, tag="rs", name=f"rsum{p}")
            for j in range(NCHUNK):
                nc.tensor.matmul(
                    out=rsum,
                    lhsT=ax[:, 128 * j : 128 * j + 128],
                    rhs=xts[:, HW + 129 * j : HW + 129 * j + 129],
                    start=(j == 0), stop=(j == NCHUNK - 1),
                )

            # softmax denominator is rsum[:, 128]; normalize
            inv = small.tile([96, 1], F32, tag="inv", name=f"inv{p}")
            nc.vector.reciprocal(out=inv, in_=rsum[0:96, 128:129])
            r2n = small.tile([96, 128], BF, tag="r2n", name=f"r2n{p}")
            nc.vector.tensor_scalar_mul(out=r2n, in0=rsum[0:96, 0:128], scalar1=inv)
            r2n_tiles[p] = r2n

        def emit_tr(p):
            """Transpose + fcl staging for pair p (deferred 2 pairs so the
            PE queue never waits on the DVE normalize)."""
            rt = sm_psum.tile([128, 96], BF, tag="sm", name=f"rt{p}")
            nc.tensor.transpose(rt, r2n_tiles[p], ident_bf[0:96, 0:96])

            # fc lhsT: fcl_all[64*hl + c, kk, 12*p + 6*b + m]
            #        = rt[64*b + c, 48*b + 12*kk + 6*hl + m]   (h = 2*kk + hl)
            rt_v = rt.rearrange("q (b kk hl m) -> q b kk hl m", b=2, kk=4, hl=2)
            for hl in range(2):
                for b in range(2):
                    dst = fcl_all_g[
                        64 * hl : 64 * hl + 64, :, 12 * p + 6 * b : 12 * p + 6 * b + 6
                    ]
                    src = rt_v[64 * b : 64 * b + 64, b, :, hl, :]
                    nc.vector.tensor_copy(out=dst, in_=src)

        # batched output projection, split in pair-halves: [96, 192] =
        # fcl^T @ Wo; half 0 (pairs 0-3) is emitted where the PE would
        # otherwise idle waiting on pair 7's exp, so only half 1 is tail.
        o2_all = o2_psum.tile([96, D], F32, tag="o2", name="o2_all")
        out96 = const.tile([BPC * M, D], F32)

        def emit_o2(h):
            # split at partition 64 (tile_position requires base 0/32/64);
            # half 0 covers pairs 0-5(+third), available once tr(5) is done
            rs = slice(0, 64) if h == 0 else slice(64, 96)
            for kk in range(4):
                nc.tensor.matmul(
                    out=o2_all[rs, :], lhsT=fcl_all_g[:, kk, rs],
                    rhs=wo_sb[:, 192 * kk : 192 * kk + 192],
                    start=(kk == 0), stop=(kk == 3),
                )
            nc.vector.tensor_add(
                out=out96[rs, :], in0=o2_all[rs, :], in1=zbo_sb[rs, :]
            )
            nc.sync.dma_start(
                out=out_h.ap()[rs.start : rs.stop, :], in_=out96[rs, :]
            )

        for p in range(NPAIR):
            emit_front(p)
            if p > 0:
                emit_av(p - 1)
            if p > 1:
                emit_tr(p - 2)
        emit_tr(NPAIR - 2)
        emit_o2(0)
        emit_av(NPAIR - 1)
        emit_tr(NPAIR - 1)
        emit_o2(1)

    return nc


def get_nc() -> bass.Bass:
    if "nc" not in _CACHE:
        nc = _build_nc()
        # The PJRT exec path serializes nc.m as-is; run Bacc's legalization
        # (wait splitting, register allocation, ...) explicitly.
        nc.finalize()
        _CACHE["nc"] = nc
    return _CACHE["nc"]


def make_in_maps(x, z, Wq, bq, Wo, bo):
    """Host-side prep + sharding into per-core input maps."""
    x = np.asarray(x, dtype=np.float32)
    z = np.asarray(z, dtype=np.float32)
    Wq = np.asarray(Wq, dtype=np.float32)
    bq = np.asarray(bq, dtype=np.float32)
    Wo = np.asarray(Wo, dtype=np.float32)
    bo = np.asarray(bo, dtype=np.float32)

    wo_bf = Wo.astype(BF16)
    # [B*M, D] batch-major rows: row 12p + 6b + m for core-local pair p
    zbo = (z + bo[None, None, :]).astype(np.float32).reshape(B * M, D)
    # pk2 = [ident 128 | wo 4*192] with wo[p, 192k+d] = Wo[128k+p, d]
    pk2 = np.zeros((128, 896), dtype=BF16)
    pk2[:, 0:128] = np.eye(128, dtype=BF16)
    pk2[:, 128:896] = np.ascontiguousarray(
        wo_bf.reshape(4, 128, D).transpose(1, 0, 2).reshape(128, 4 * D)
    )

    # Global fp8 image of x: per pair-row [xf 3136 | x^T-chunks 25*129].
    # x^T block j covers n0=128j (j<24) or n0=3008 (j=24, overlapped); col
    # 128 of each block is the softmax-denominator ones column.
    XROW = HW + 129 * NCHUNK
    x8 = x.astype(F8)
    xx_glob = np.zeros((B // 2, 128, XROW), dtype=F8)
    xf_glob = xx_glob[:, :, 0:HW]
    xf_glob[...] = x8.reshape(B // 2, 128, HW)
    one8 = np.ones((), dtype=F8)
    for j in range(NCHUNK):
        n0 = 128 * j if j < NCHUNK - 1 else HW - 128
        blk = xf_glob[:, :, n0 : n0 + 128]  # [P, c2, 128]
        xx_glob[:, :, HW + 129 * j : HW + 129 * j + 128] = blk.transpose(0, 2, 1)
        xx_glob[:, :, HW + 129 * j + 128] = one8
    # zero the rows of the overlapped chunk that duplicate chunk 23
    xx_glob[:, 0:64, HW + 129 * (NCHUNK - 1) : HW + 129 * (NCHUNK - 1) + 129] = 0

    # Block-diagonal q^T images (scale is applied in the on-device exp).
    # q row for (h, m) is the flat row u = 6h + m of (z@Wq + bq).reshape(48, 64)
    qr = (z.reshape(B * M, D) @ Wq + bq).reshape(B // 2, 2, 48, 64)
    qt_glob = np.zeros((B // 2, 128, 96), dtype=F8)
    for bb in range(2):
        qt_glob[:, 64 * bb : 64 * bb + 64, 48 * bb : 48 * bb + 48] = (
            qr[:, bb].transpose(0, 2, 1).astype(F8)
        )

    in_maps = []
    for i in range(N_CORES):
        ps = slice(i * NPAIR, (i + 1) * NPAIR)
        # [128, NPAIR*96] per core: col 96p + 48b + u
        qt8 = np.ascontiguousarray(
            qt_glob[ps].transpose(1, 0, 2).reshape(128, NPAIR * 96)
        )
        in_maps.append(
            {
                "xx8": np.ascontiguousarray(xx_glob[ps]),
                "qt8": qt8,
                "zbo": zbo[i * BPC * M : (i + 1) * BPC * M],
                "pk2": pk2,
            }
        )
    return in_maps


def kernel(**inputs) -> np.ndarray:
    nc = get_nc()
    in_maps = make_in_maps(
        inputs["x"], inputs["z"], inputs["Wq"], inputs["bq"],
        inputs["Wo"], inputs["bo"],
    )
    res = run_bass_kernel_spmd(nc, in_maps, list(range(N_CORES)))
    out = np.concatenate(
        [np.asarray(res.results[i]["out"]).reshape(BPC, M, D) for i in range(N_CORES)],
        axis=0,
    )
    return out.astype(np.float32)
